# revision 9
# baseline (speedup 1.0000x reference)
"""Bahdanau-attention kernel for trn2, data-parallel over batch across 8 cores.

Per-core computation (B_LOC = 4 batches, S = 4096, H = E = 256):
  energy = tanh(hidden @ Wh.T + enc @ We.T + b_attn)      [b, s, e]
  scores = energy . v                                      [b, s]
  attn   = softmax(scores) over s  (no max-subtraction: scores bounded by ||v||_1)
  out    = sum_s attn * enc                                [b, h]

Design:
  - enc slice read from HBM once (4KB contiguous runs), cast to bf16 in the
    DMA -> x_res resident in SBUF; per-group xbar transposes produce X^T for
    the energy matmul (h on partitions).
  - The Tile scheduler models all DMA transfers as one exclusive device and
    emits semaphore chains that serialize HBM reads against the SBUF->SBUF
    xbar transposes (measured: perfectly serial, 88us wall). The hardware can
    overlap them (verified: 36us overlap under an ASAP schedule). _dma_surgery
    rewrites the DMA waits post-schedule: reads wait on nothing, transposes
    wait only on their own group's read. Completion order on every semaphore
    lane is unchanged, so all downstream waits stay valid.
  - Energy matmuls run in per-eh waves with the stationary We^T chunk reused
    across the 4 batches; tanh folds the per-partition bias
    qb[e] = hidden @ Wh.T + b_attn; the v-dot runs on the PE with v
    stationary, landing all 4 batches' score strips in one PSUM bank at
    partitions {0,32,64,96} (tile_position col packing -> concurrent MMs).
  - One Exp activation per group produces exp(scores) and the softmax
    denominators (accum_out); exp strips are PE-transposed to [s-part, b];
    the unnormalized context accumulates in a single [4, 1024] PSUM tile
    (M=4 matmuls vs resident native X; off-diagonal (b,b') blocks discarded).
  - softmax normalization (divide by denominator) happens on the host.
"""

import numpy as np

B, S, H = 32, 4096, 256
NCORES = 8
BL = B // NCORES  # batches per core
NG = 8            # s-groups of 512 rows
E = H

_CACHE = {}


def _split_multiwait(nc, mybir):
    """This walrus/ISA build allows ONE sync-wait slot per instruction.
    Move extra waits onto same-engine NoOps inserted just before."""
    for blk in nc.m.functions[0].blocks:
        insts = blk.instructions
        out = []
        changed = False
        for inst in insts:
            si = inst.sync_info
            waits = list(si.on_wait) if si is not None else []
            if len(waits) > 1:
                for w in waits[:-1]:
                    nop = mybir.InstNoOp(
                        name=nc.get_next_instruction_name(), ins=[], outs=[]
                    )
                    nop.engine = inst.engine
                    nop.sync_info = mybir.SyncInfo(on_wait=[w], on_update=[])
                    out.append(nop)
                inst.sync_info = mybir.SyncInfo(
                    on_wait=[waits[-1]], on_update=list(si.on_update)
                )
                changed = True
            out.append(inst)
        if changed:
            insts[:] = out


def _dma_surgery(nc, mybir, read_names, tpose_deps, setup_read_names=(),
                 pace_deps=None, verbose=False):
    """Break the scheduler's false serialization between the SWDGE HBM reads
    and the HWDGE xbar transposes.

    Safety argument: reads write fresh x_res tiles (no reuse), transposes
    write fresh xt tiles (bufs == NG, no reuse), and both streams execute in
    unchanged FIFO order on their own queues, so every DMA lane semaphore
    still reaches each value in the same order as the legacy schedule — only
    earlier. All remaining waits are sem-ge, hence monotone-safe.
    """
    blocks = nc.m.functions[0].blocks
    insts = {}
    order = []
    for blk in blocks:
        for i in blk.instructions:
            insts[i.name] = i
            order.append(i)

    # Per-DMA completion signature: (update, cumulative value) in BIR order.
    cum = {}
    read_sig = {}
    tpose_sig = {}
    rset = set(read_names)
    tset_all = {tn for tn, _ in tpose_deps}
    for i in order:
        si = i.sync_info
        if si is None:
            continue
        for u in si.on_update:
            if not (u.ant_name.startswith("DMASW") or u.ant_name.startswith("DMAHW")):
                continue
            cum[u.ant_name] = cum.get(u.ant_name, 0) + u.update_value
            if i.name in rset:
                read_sig[i.name] = (u, cum[u.ant_name])
            if i.name in tset_all:
                tpose_sig[i.name] = (u, cum[u.ant_name])
    missing = [rn for rn in read_names if rn not in read_sig]
    assert not missing, f"reads without DMA-lane updates: {missing}"

    def is_lane(w):
        return w.ant_name.startswith("DMASW") or w.ant_name.startswith("DMAHW")

    # 1. Reads wait on nothing DMA-related (GpSimd FIFO + SWDGE ring throttle).
    for rn in list(read_names) + list(setup_read_names):
        i = insts[rn]
        si = i.sync_info
        if si is None:
            continue
        keep = [w for w in si.on_wait if not is_lane(w)]
        if verbose and len(keep) != len(si.on_wait):
            print(f"  read {rn}: dropped {len(si.on_wait) - len(keep)} lane waits")
        i.sync_info = mybir.SyncInfo(on_wait=keep, on_update=list(si.on_update))

    # 1b. Pacing: a read may start only once the transpose two groups back
    #     has completed, so at most one HBM read competes with each xbar
    #     transpose for SDMA packet slots (reads' ~8x larger packets would
    #     otherwise starve the transposes that gate compute).
    for rn, tns in (pace_deps or {}).items():
        i = insts[rn]
        si = i.sync_info
        waits = list(si.on_wait) if si else []
        for tn in tns:
            u, val = tpose_sig[tn]
            waits.append(
                mybir.SyncWait(
                    sync_type="semaphore",
                    id=u.id,
                    ant_name=u.ant_name,
                    wait_mode="sem-ge-imm",
                    wait_value=val,
                )
            )
        if verbose:
            print(f"  pace read {rn}: += {[(t) for t in tns]}")
        i.sync_info = mybir.SyncInfo(
            on_wait=waits, on_update=list(si.on_update) if si else []
        )

    # 2. Transposes wait exactly on their group's read completion.
    for tn, deps in tpose_deps:
        i = insts[tn]
        si = i.sync_info
        waits = []
        for rn in deps:
            u, val = read_sig[rn]
            waits.append(
                mybir.SyncWait(
                    sync_type="semaphore",
                    id=u.id,
                    ant_name=u.ant_name,
                    wait_mode="sem-ge-imm",
                    wait_value=val,
                )
            )
        if verbose:
            old = [f"{w.ant_name}>={w.wait_value}" for w in (si.on_wait if si else [])]
            new = [f"{w.ant_name}>={w.wait_value}" for w in waits]
            print(f"  tpose {tn}: {old} -> {new}")
        i.sync_info = mybir.SyncInfo(
            on_wait=waits, on_update=list(si.on_update) if si else []
        )

    # 3. Scheduler-issued SP-side read waits (now redundant: each transpose
    #    carries its own) would head-of-line block the Sync FIFO; drop them.
    tile_blocks = [
        b
        for b in blocks
        if b.name.startswith("tile_context") and not b.name.endswith("_end")
    ]
    tset = {tn for tn, _ in tpose_deps}
    for blk in tile_blocks:
        for i in blk.instructions:
            if i.engine != mybir.EngineType.SP or i.name in tset:
                continue
            if isinstance(i, (mybir.InstDMACopy, mybir.InstDmaTransposeAnt)):
                continue
            si = i.sync_info
            if si is None or not si.on_wait:
                continue
            if all(w.ant_name.startswith("DMASW") for w in si.on_wait):
                if verbose:
                    print(
                        f"  SP {type(i).__name__} {i.name}: dropped "
                        f"{[w.ant_name + '>=' + str(w.wait_value) for w in si.on_wait]}"
                    )
                i.sync_info = mybir.SyncInfo(on_wait=[], on_update=list(si.on_update))


def _build(verbose=False):
    import concourse.bass as bass
    import concourse.tile as tile
    from concourse import mybir
    from concourse.masks import make_identity

    f32 = mybir.dt.float32
    bf16 = mybir.dt.bfloat16
    AF = mybir.ActivationFunctionType

    nc = bass.Bass(num_swdge_queues=2, dynamic_dma_scratch_size=65536)
    hid_t = nc.dram_tensor("hidden", [BL, H], f32, kind="ExternalInput")
    enc_t = nc.dram_tensor("enc", [S, BL, H], f32, kind="ExternalInput")
    wat_t = nc.dram_tensor("w_attn", [H, 2 * H], f32, kind="ExternalInput")
    bat_t = nc.dram_tensor("b_attn", [H], f32, kind="ExternalInput")
    wv_t = nc.dram_tensor("w_v", [1, H], f32, kind="ExternalInput")
    # unnormalized context halves + denominators; normalized on host
    ctxu_t = nc.dram_tensor("ctxu", [2, 2, 512], f32, kind="ExternalOutput")
    den_t = nc.dram_tensor("den", [97, 1], f32, kind="ExternalOutput")

    hid = hid_t.ap()
    enc = enc_t.ap()
    wat = wat_t.ap()
    bat = bat_t.ap().rearrange("(o c) -> o c", o=1)  # [1, 256]
    wv = wv_t.ap()

    read_names = []        # instruction names of the SWDGE x_res reads
    setup_read_names = []  # HWDGE setup reads (strip serial-chain waits only)
    read_of_group = {}     # g -> [read names]
    tpose_deps = []        # (transpose name, [read names])
    tpose_of_group = {}    # g -> [transpose names]

    with tile.TileContext(nc) as tc:
        with (
            tc.tile_pool(name="const", bufs=1) as cp,
            tc.tile_pool(name="xres", bufs=1) as xrp,
            tc.tile_pool(name="xtp", bufs=8) as xtp,
            tc.tile_pool(name="thp", bufs=8) as thp,
            tc.tile_pool(name="stat", bufs=1) as stp,
            tc.tile_pool(name="misc", bufs=2) as wp,
            tc.tile_pool(name="pe", bufs=4, space="PSUM") as ppe,
            tc.tile_pool(name="ps", bufs=2, space="PSUM") as pps,
            tc.tile_pool(name="pc", bufs=1, space="PSUM") as ppc,
        ):
            st_g = [
                stp.tile([97, 512], bf16, tag=f"st{g}", name=f"st{g}")
                for g in range(NG)
            ]
            for g in range(NG):
                nc.vector.memset(st_g[g], 0.0)

            ident = cp.tile([128, 128], f32)
            make_identity(nc, ident)
            ident16 = cp.tile([128, 128], bf16)
            nc.vector.tensor_copy(out=ident16, in_=ident)

            # ---------- resident enc: bf16 cast-DMA reads ----------
            # One DMA per (group, batch): 512B destination runs, so the read
            # packets are comparable in size to the xbar transpose packets and
            # the SDMA round-robin shares bandwidth fairly between the two
            # streams (2KB read runs would starve the transposes ~85/15).
            x_res = []
            for g in range(NG):
                t = xrp.tile([128, 4, 4 * H], bf16, tag=f"xr{g}", name=f"xr{g}")
                src = enc[g * 512 : (g + 1) * 512, :, :].rearrange(
                    "(jl p) b h -> p jl b h", p=128
                )
                names = []
                if g == 0:
                    # split by jl-half too so the first transpose starts sooner
                    for half in range(2):
                        for b in range(BL):
                            r = nc.gpsimd.dma_start(
                                out=t[:, 2 * half : 2 * half + 2, b * H : (b + 1) * H],
                                in_=src[:, 2 * half : 2 * half + 2, b],
                            )
                            names.append(r.ins.name)
                else:
                    for b in range(BL):
                        r = nc.gpsimd.dma_start(
                            out=t[:, :, b * H : (b + 1) * H], in_=src[:, :, b]
                        )
                        names.append(r.ins.name)
                read_names += names
                read_of_group[g] = names
                x_res.append(t)

            u_g = [
                stp.tile([128, BL, 4], bf16, tag=f"ug{g}", name=f"ug{g}")
                for g in range(NG)
            ]
            acc_all = stp.tile([97, NG], f32)
            wet16 = [cp.tile([128, E], bf16, tag=f"wet{i}", name=f"wet{i}") for i in range(2)]
            qb = [cp.tile([128, BL], f32, tag=f"qb{i}", name=f"qb{i}") for i in range(2)]
            vt16 = [cp.tile([128, 1], bf16, tag=f"vt{i}", name=f"vt{i}") for i in range(2)]

            # ---------------- setup: weights / q / v ----------------
            with tc.tile_pool(name="setsb", bufs=1) as ssb:
                w_nat = [
                    ssb.tile([128, 2 * H], f32, tag="wn", name=f"wn{i}")
                    for i in range(2)
                ]
                for eh in range(2):
                    rw = nc.sync.dma_start(
                        out=w_nat[eh], in_=wat[eh * 128 : (eh + 1) * 128, :]
                    )
                    setup_read_names.append(rw.ins.name)
                b_attn_sb = ssb.tile([1, H], f32)
                rb = nc.sync.dma_start(out=b_attn_sb, in_=bat)
                setup_read_names.append(rb.ins.name)
                v_sb = ssb.tile([1, H], f32)
                rv = nc.sync.dma_start(out=v_sb, in_=wv)
                setup_read_names.append(rv.ins.name)
                h_nat = ssb.tile([BL, H], f32)
                rh = nc.sync.dma_start(out=h_nat, in_=hid)
                setup_read_names.append(rh.ins.name)
                ones4 = ssb.tile([1, BL], f32)
                nc.vector.memset(ones4, 1.0)

                wht = [
                    ssb.tile([128, E], f32, tag=f"wht{i}", name=f"wht{i}")
                    for i in range(2)
                ]
                for eh in range(2):
                    for cblk in range(4):  # column blocks of W_attn
                        pt = pps.tile([128, 128], f32, tag="s", bufs=2, name="pt_w")
                        nc.tensor.transpose(
                            pt, w_nat[eh][:, cblk * 128 : (cblk + 1) * 128], ident
                        )
                        if cblk < 2:  # Wh columns
                            nc.scalar.copy(
                                out=wht[cblk][:, eh * 128 : (eh + 1) * 128], in_=pt
                            )
                        else:  # We columns
                            nc.scalar.copy(
                                out=wet16[cblk - 2][:, eh * 128 : (eh + 1) * 128],
                                in_=pt,
                            )

                ht = [
                    ssb.tile([128, BL], f32, tag=f"ht{i}", name=f"ht{i}")
                    for i in range(2)
                ]
                for hh in range(2):
                    pt = pps.tile([128, 128], f32, tag="s", bufs=2, name="pt_h")
                    nc.tensor.transpose(
                        pt[:, :BL], h_nat[:, hh * 128 : (hh + 1) * 128], ident[:BL, :BL]
                    )
                    nc.scalar.copy(out=ht[hh], in_=pt[:, :BL])

                for eh in range(2):
                    pt = pps.tile([128, 128], f32, tag="s", bufs=2, name="pt_v")
                    nc.tensor.transpose(
                        pt[:, :1], v_sb[:, eh * 128 : (eh + 1) * 128], ident[:1, :1]
                    )
                    nc.scalar.copy(out=vt16[eh], in_=pt[:, :1])

                # qb[eh][e, b] = sum_h WhT[h, e] * hT[h, b] + b_attn[e]
                for eh in range(2):
                    pq = pps.tile([128, 128], f32, tag="s", bufs=2, name="pt_q")
                    for hh in range(2):
                        nc.tensor.matmul(
                            pq[:, :BL],
                            wht[hh][:, eh * 128 : (eh + 1) * 128],
                            ht[hh],
                            start=(hh == 0),
                            stop=False,
                        )
                    nc.tensor.matmul(
                        pq[:, :BL],
                        b_attn_sb[:, eh * 128 : (eh + 1) * 128],
                        ones4,
                        start=False,
                        stop=True,
                    )
                    nc.scalar.copy(out=qb[eh], in_=pq[:, :BL])

            # ---------------- main loop ----------------
            pctx = [
                ppc.tile([2, 512], f32, tag=f"ctx{h}", name=f"pctx{h}")
                for h in range(2)
            ]

            def ctx_group(g):
                for half in range(2):
                    for jl in range(4):
                        n = g * 4 + jl
                        nc.tensor.matmul(
                            pctx[half],
                            u_g[g][:, 2 * half : 2 * half + 2, jl],
                            x_res[g][:, jl, half * 512 : (half + 1) * 512],
                            start=(n == 0),
                            stop=(n == NG * 4 - 1),
                        )

            for g in range(NG):
                xt_t = xtp.tile([128, 4096], bf16, tag="xt", name="xt")
                if g == 0:
                    for half in range(2):
                        tp = nc.sync.dma_start_transpose(
                            xt_t[:, half * 2048 : (half + 1) * 2048].rearrange(
                                "p (grp s) -> p grp s", s=128
                            ),
                            x_res[g][:, 2 * half : 2 * half + 2].rearrange(
                                "p a q -> p (a q)"
                            ),
                        )
                        tpose_deps.append(
                            (tp.ins.name, read_of_group[g][half * BL : (half + 1) * BL])
                        )
                        tpose_of_group.setdefault(g, []).append(tp.ins.name)
                else:
                    tp = nc.sync.dma_start_transpose(
                        xt_t.rearrange("p (grp s) -> p grp s", s=128),
                        x_res[g].rearrange("p a q -> p (a q)"),
                    )
                    tpose_deps.append((tp.ins.name, read_of_group[g]))
                    tpose_of_group.setdefault(g, []).append(tp.ins.name)
                xt_v = xt_t.rearrange("p (jl c s) -> p c jl s", jl=4, s=128)

                # context for the previous group first: its data is ready, so
                # it never head-of-line blocks the Tensor FIFO.
                if g >= 1:
                    ctx_group(g - 1)

                strip = pps.tile([97, 512], f32, tag="s", name="strip")
                for eh in range(2):
                    pe_t = [
                        ppe.tile([128, 512], f32, tag="e", name=f"pe{b}")
                        for b in range(BL)
                    ]
                    # stationary We^T chunk reused across the 4 batches
                    for hh in range(2):
                        for b in range(BL):
                            nc.tensor.matmul(
                                pe_t[b],
                                wet16[hh][:, eh * 128 : (eh + 1) * 128],
                                xt_v[:, b * 2 + hh],
                                start=(hh == 0),
                                stop=(hh == 1),
                            )
                    th_eh = []
                    for b in range(BL):
                        th = thp.tile([128, 512], bf16, tag="th", name="th")
                        nc.scalar.activation(
                            out=th,
                            in_=pe_t[b],
                            func=AF.Tanh,
                            bias=qb[eh][:, b : b + 1],
                        )
                        th_eh.append(th)
                    for b in range(BL):
                        nc.tensor.matmul(
                            strip[32 * b : 32 * b + 1, :],
                            vt16[eh],
                            th_eh[b],
                            start=(eh == 0),
                            stop=(eh == 1),
                            tile_position=(0, 32 * b),
                        )

                nc.scalar.activation(
                    out=st_g[g],
                    in_=strip,
                    func=AF.Exp,
                    accum_out=acc_all[:, g : g + 1],
                )
                for c in range(4):
                    pt = pps.tile([128, 256], bf16, tag="s", bufs=2, name="pt_u")
                    nc.tensor.transpose(
                        pt[:, :97],
                        st_g[g][:, c * 128 : (c + 1) * 128],
                        ident16[:97, :97],
                    )
                    nc.vector.tensor_copy(
                        out=u_g[g][:, :, c],
                        in_=pt.rearrange("p (a r) -> p a r", r=32)[:, :4, 0],
                    )

            ctx_group(NG - 1)
            for half in range(2):
                csb = wp.tile([2, 512], f32, tag="csb", name=f"csb{half}")
                nc.scalar.copy(out=csb, in_=pctx[half])
                nc.sync.dma_start(out=ctxu_t.ap()[half], in_=csb)

            accs = wp.tile([97, 1], f32)
            nc.vector.reduce_sum(out=accs, in_=acc_all, axis=mybir.AxisListType.X)
            nc.sync.dma_start(out=den_t.ap(), in_=accs)

    _dma_surgery(
        nc, mybir, read_names, tpose_deps, setup_read_names, verbose=verbose,
    )
    _split_multiwait(nc, mybir)
    return nc


def kernel(**inputs):
    from concourse.bass_utils import run_bass_kernel_spmd

    hidden = np.asarray(inputs["hidden"], dtype=np.float32)
    enc = np.asarray(inputs["encoder_outputs"], dtype=np.float32)
    w_attn = np.ascontiguousarray(np.asarray(inputs["W_attn"], dtype=np.float32))
    b_attn = np.ascontiguousarray(np.asarray(inputs["b_attn"], dtype=np.float32))
    w_v = np.ascontiguousarray(np.asarray(inputs["W_v"], dtype=np.float32))

    if "nc" not in _CACHE:
        _CACHE["nc"] = _build()
    nc = _CACHE["nc"]

    in_maps = []
    for c in range(NCORES):
        sl = slice(c * BL, (c + 1) * BL)
        in_maps.append(
            {
                "hidden": np.ascontiguousarray(hidden[sl]),
                "enc": np.ascontiguousarray(enc[:, sl, :]),
                "w_attn": w_attn,
                "b_attn": b_attn,
                "w_v": w_v,
            }
        )

    trace = bool(_CACHE.get("trace", False))
    res = run_bass_kernel_spmd(nc, in_maps, core_ids=list(range(NCORES)), trace=trace)
    _CACHE["last_results"] = res

    out = np.empty((1, B, H), dtype=np.float32)
    for c in range(NCORES):
        ctxu = res.results[c]["ctxu"]  # [2, 2, 512]
        den = res.results[c]["den"]    # [97, 1]
        for b in range(BL):
            half, row = b // 2, b % 2
            vals = ctxu[half, row, row * 256 : row * 256 + 256]
            out[0, c * BL + b] = vals / den[32 * b, 0]
    return out


# revision 13
# speedup vs baseline: 1.1605x; 1.1605x over previous
"""Bahdanau-attention kernel for trn2, data-parallel over batch across 8 cores.

Per-core computation (B_LOC = 4 batches, S = 4096, H = E = 256):
  energy = tanh(hidden @ Wh.T + enc @ We.T + b_attn)      [b, s, e]
  scores = energy . v                                      [b, s]
  attn   = softmax(scores) over s  (no max-subtraction: scores bounded by ||v||_1)
  out    = sum_s attn * enc                                [b, h]

Design:
  - enc slice read from HBM once (4KB contiguous runs), cast to bf16 in the
    DMA -> x_res resident in SBUF; per-group xbar transposes produce X^T for
    the energy matmul (h on partitions).
  - The Tile scheduler models all DMA transfers as one exclusive device and
    emits semaphore chains that serialize HBM reads against the SBUF->SBUF
    xbar transposes (measured: perfectly serial, 88us wall). The hardware can
    overlap them (verified: 36us overlap under an ASAP schedule). _dma_surgery
    rewrites the DMA waits post-schedule: reads wait on nothing, transposes
    wait only on their own group's read. Completion order on every semaphore
    lane is unchanged, so all downstream waits stay valid.
  - Energy matmuls run in per-eh waves with the stationary We^T chunk reused
    across the 4 batches; tanh folds the per-partition bias
    qb[e] = hidden @ Wh.T + b_attn; the v-dot runs on the PE with v
    stationary, landing all 4 batches' score strips in one PSUM bank at
    partitions {0,32,64,96} (tile_position col packing -> concurrent MMs).
  - One Exp activation per group produces exp(scores) and the softmax
    denominators (accum_out); exp strips are PE-transposed to [s-part, b];
    the unnormalized context accumulates in a single [4, 1024] PSUM tile
    (M=4 matmuls vs resident native X; off-diagonal (b,b') blocks discarded).
  - softmax normalization (divide by denominator) happens on the host.
"""

import numpy as np

B, S, H = 32, 4096, 256
NCORES = 8
BL = B // NCORES  # batches per core
NG = 8            # s-groups of 512 rows
E = H

_CACHE = {}


def _split_multiwait(nc, mybir):
    """This walrus/ISA build allows ONE sync-wait slot per instruction.
    Move extra waits onto same-engine NoOps inserted just before."""
    for blk in nc.m.functions[0].blocks:
        insts = blk.instructions
        out = []
        changed = False
        for inst in insts:
            si = inst.sync_info
            waits = list(si.on_wait) if si is not None else []
            if len(waits) > 1:
                for w in waits[:-1]:
                    nop = mybir.InstNoOp(
                        name=nc.get_next_instruction_name(), ins=[], outs=[]
                    )
                    nop.engine = inst.engine
                    nop.sync_info = mybir.SyncInfo(on_wait=[w], on_update=[])
                    out.append(nop)
                inst.sync_info = mybir.SyncInfo(
                    on_wait=[waits[-1]], on_update=list(si.on_update)
                )
                changed = True
            out.append(inst)
        if changed:
            insts[:] = out


def _dma_surgery(nc, mybir, read_names, tpose_deps, setup_read_names=(),
                 pace_deps=None, verbose=False):
    """Break the scheduler's false serialization between the SWDGE HBM reads
    and the HWDGE xbar transposes.

    Safety argument: reads write fresh x_res tiles (no reuse), transposes
    write fresh xt tiles (bufs == NG, no reuse), and both streams execute in
    unchanged FIFO order on their own queues, so every DMA lane semaphore
    still reaches each value in the same order as the legacy schedule — only
    earlier. All remaining waits are sem-ge, hence monotone-safe.
    """
    blocks = nc.m.functions[0].blocks
    insts = {}
    order = []
    for blk in blocks:
        for i in blk.instructions:
            insts[i.name] = i
            order.append(i)

    # Per-DMA completion signature: (update, cumulative value) in BIR order.
    cum = {}
    read_sig = {}
    tpose_sig = {}
    rset = set(read_names)
    tset_all = {tn for tn, _ in tpose_deps}
    for i in order:
        si = i.sync_info
        if si is None:
            continue
        for u in si.on_update:
            if not (u.ant_name.startswith("DMASW") or u.ant_name.startswith("DMAHW")):
                continue
            cum[u.ant_name] = cum.get(u.ant_name, 0) + u.update_value
            if i.name in rset:
                read_sig[i.name] = (u, cum[u.ant_name])
            if i.name in tset_all:
                tpose_sig[i.name] = (u, cum[u.ant_name])
    missing = [rn for rn in read_names if rn not in read_sig]
    assert not missing, f"reads without DMA-lane updates: {missing}"

    def is_lane(w):
        return w.ant_name.startswith("DMASW") or w.ant_name.startswith("DMAHW")

    # 1. Reads wait on nothing DMA-related (GpSimd FIFO + SWDGE ring throttle).
    for rn in list(read_names) + list(setup_read_names):
        i = insts[rn]
        si = i.sync_info
        if si is None:
            continue
        keep = [w for w in si.on_wait if not is_lane(w)]
        if verbose and len(keep) != len(si.on_wait):
            print(f"  read {rn}: dropped {len(si.on_wait) - len(keep)} lane waits")
        i.sync_info = mybir.SyncInfo(on_wait=keep, on_update=list(si.on_update))

    # 1b. Pacing: a read may start only once the transpose two groups back
    #     has completed, so at most one HBM read competes with each xbar
    #     transpose for SDMA packet slots (reads' ~8x larger packets would
    #     otherwise starve the transposes that gate compute).
    for rn, tns in (pace_deps or {}).items():
        i = insts[rn]
        si = i.sync_info
        waits = list(si.on_wait) if si else []
        for tn in tns:
            u, val = tpose_sig[tn]
            waits.append(
                mybir.SyncWait(
                    sync_type="semaphore",
                    id=u.id,
                    ant_name=u.ant_name,
                    wait_mode="sem-ge-imm",
                    wait_value=val,
                )
            )
        if verbose:
            print(f"  pace read {rn}: += {[(t) for t in tns]}")
        i.sync_info = mybir.SyncInfo(
            on_wait=waits, on_update=list(si.on_update) if si else []
        )

    # 2. Transposes wait exactly on their group's read completion.
    for tn, deps in tpose_deps:
        i = insts[tn]
        si = i.sync_info
        waits = []
        for rn in deps:
            u, val = read_sig[rn]
            waits.append(
                mybir.SyncWait(
                    sync_type="semaphore",
                    id=u.id,
                    ant_name=u.ant_name,
                    wait_mode="sem-ge-imm",
                    wait_value=val,
                )
            )
        if verbose:
            old = [f"{w.ant_name}>={w.wait_value}" for w in (si.on_wait if si else [])]
            new = [f"{w.ant_name}>={w.wait_value}" for w in waits]
            print(f"  tpose {tn}: {old} -> {new}")
        i.sync_info = mybir.SyncInfo(
            on_wait=waits, on_update=list(si.on_update) if si else []
        )

    # 3. Scheduler-issued SP-side read waits (now redundant: each transpose
    #    carries its own) would head-of-line block the Sync FIFO; drop them.
    tile_blocks = [
        b
        for b in blocks
        if b.name.startswith("tile_context") and not b.name.endswith("_end")
    ]
    tset = {tn for tn, _ in tpose_deps}
    for blk in tile_blocks:
        for i in blk.instructions:
            if i.engine != mybir.EngineType.SP or i.name in tset:
                continue
            if isinstance(i, (mybir.InstDMACopy, mybir.InstDmaTransposeAnt)):
                continue
            si = i.sync_info
            if si is None or not si.on_wait:
                continue
            if all(w.ant_name.startswith("DMASW") for w in si.on_wait):
                if verbose:
                    print(
                        f"  SP {type(i).__name__} {i.name}: dropped "
                        f"{[w.ant_name + '>=' + str(w.wait_value) for w in si.on_wait]}"
                    )
                i.sync_info = mybir.SyncInfo(on_wait=[], on_update=list(si.on_update))


def _build(verbose=False):
    import concourse.bass as bass
    import concourse.tile as tile
    from concourse import mybir
    from concourse.masks import make_identity

    f32 = mybir.dt.float32
    bf16 = mybir.dt.bfloat16
    AF = mybir.ActivationFunctionType

    nc = bass.Bass(num_swdge_queues=2, dynamic_dma_scratch_size=65536)
    hid_t = nc.dram_tensor("hidden", [BL, H], f32, kind="ExternalInput")
    enc_t = nc.dram_tensor("enc", [S, BL, H], f32, kind="ExternalInput")
    wat_t = nc.dram_tensor("w_attn", [H, 2 * H], f32, kind="ExternalInput")
    bat_t = nc.dram_tensor("b_attn", [H], f32, kind="ExternalInput")
    wv_t = nc.dram_tensor("w_v", [1, H], f32, kind="ExternalInput")
    # unnormalized context halves + denominators; normalized on host
    ctxu_t = nc.dram_tensor("ctxu", [2, 2, 512], f32, kind="ExternalOutput")
    den_t = nc.dram_tensor("den", [97, 1], f32, kind="ExternalOutput")

    hid = hid_t.ap()
    enc = enc_t.ap()
    wat = wat_t.ap()
    bat = bat_t.ap().rearrange("(o c) -> o c", o=1)  # [1, 256]
    wv = wv_t.ap()

    read_names = []        # instruction names of the SWDGE x_res reads
    setup_read_names = []  # HWDGE setup reads (strip serial-chain waits only)
    read_of_group = {}     # g -> [read names]
    tpose_deps = []        # (transpose name, [read names])
    tpose_of_group = {}    # g -> [transpose names]

    with tile.TileContext(nc) as tc:
        with (
            tc.tile_pool(name="const", bufs=1) as cp,
            tc.tile_pool(name="xres", bufs=1) as xrp,
            tc.tile_pool(name="xtp", bufs=8) as xtp,
            tc.tile_pool(name="thp", bufs=8) as thp,
            tc.tile_pool(name="stat", bufs=1) as stp,
            tc.tile_pool(name="misc", bufs=2) as wp,
            tc.tile_pool(name="pe", bufs=4, space="PSUM") as ppe,
            tc.tile_pool(name="ps", bufs=2, space="PSUM") as pps,
            tc.tile_pool(name="pc", bufs=1, space="PSUM") as ppc,
        ):
            st_g = [
                stp.tile([97, 512], bf16, tag=f"st{g}", name=f"st{g}")
                for g in range(NG)
            ]
            for g in range(NG):
                nc.vector.memset(st_g[g], 0.0)

            ident = cp.tile([128, 128], f32)
            make_identity(nc, ident)
            ident16 = cp.tile([128, 128], bf16)
            nc.vector.tensor_copy(out=ident16, in_=ident)

            # ---------- resident enc: bf16 cast-DMA reads ----------
            x_res = []
            for g in range(NG):
                t = xrp.tile([128, 4, 4 * H], bf16, tag=f"xr{g}", name=f"xr{g}")
                src = enc[g * 512 : (g + 1) * 512, :, :].rearrange(
                    "(jl p) b h -> p jl (b h)", p=128
                )
                if g == 0:
                    # two 1-MiB halves so the first transpose starts sooner
                    r0 = nc.gpsimd.dma_start(out=t[:, 0:2], in_=src[:, 0:2])
                    r1 = nc.gpsimd.dma_start(out=t[:, 2:4], in_=src[:, 2:4])
                    read_names += [r0.ins.name, r1.ins.name]
                    read_of_group[g] = [r0.ins.name, r1.ins.name]
                else:
                    r = nc.gpsimd.dma_start(out=t, in_=src)
                    read_names.append(r.ins.name)
                    read_of_group[g] = [r.ins.name]
                x_res.append(t)

            u_g = [
                stp.tile([128, BL, 4], bf16, tag=f"ug{g}", name=f"ug{g}")
                for g in range(NG)
            ]
            acc_all = stp.tile([97, NG], f32)
            wet16 = [cp.tile([128, E], bf16, tag=f"wet{i}", name=f"wet{i}") for i in range(2)]
            qb = [cp.tile([128, BL], f32, tag=f"qb{i}", name=f"qb{i}") for i in range(2)]
            vt16 = [cp.tile([128, 1], bf16, tag=f"vt{i}", name=f"vt{i}") for i in range(2)]

            # ---------------- setup: weights / q / v ----------------
            with tc.tile_pool(name="setsb", bufs=1) as ssb:
                w_nat = [
                    ssb.tile([128, 2 * H], f32, tag="wn", name=f"wn{i}")
                    for i in range(2)
                ]
                for eh in range(2):
                    rw = nc.sync.dma_start(
                        out=w_nat[eh], in_=wat[eh * 128 : (eh + 1) * 128, :]
                    )
                    setup_read_names.append(rw.ins.name)
                b_attn_sb = ssb.tile([1, H], f32)
                rb = nc.sync.dma_start(out=b_attn_sb, in_=bat)
                setup_read_names.append(rb.ins.name)
                v_sb = ssb.tile([1, H], f32)
                rv = nc.sync.dma_start(out=v_sb, in_=wv)
                setup_read_names.append(rv.ins.name)
                h_nat = ssb.tile([BL, H], f32)
                rh = nc.sync.dma_start(out=h_nat, in_=hid)
                setup_read_names.append(rh.ins.name)
                ones4 = ssb.tile([1, BL], f32)
                nc.vector.memset(ones4, 1.0)

                wht = [
                    ssb.tile([128, E], f32, tag=f"wht{i}", name=f"wht{i}")
                    for i in range(2)
                ]
                for eh in range(2):
                    for cblk in range(4):  # column blocks of W_attn
                        pt = pps.tile([128, 128], f32, tag="s", bufs=2, name="pt_w")
                        nc.tensor.transpose(
                            pt, w_nat[eh][:, cblk * 128 : (cblk + 1) * 128], ident
                        )
                        if cblk < 2:  # Wh columns
                            nc.scalar.copy(
                                out=wht[cblk][:, eh * 128 : (eh + 1) * 128], in_=pt
                            )
                        else:  # We columns
                            nc.scalar.copy(
                                out=wet16[cblk - 2][:, eh * 128 : (eh + 1) * 128],
                                in_=pt,
                            )

                ht = [
                    ssb.tile([128, BL], f32, tag=f"ht{i}", name=f"ht{i}")
                    for i in range(2)
                ]
                for hh in range(2):
                    pt = pps.tile([128, 128], f32, tag="s", bufs=2, name="pt_h")
                    nc.tensor.transpose(
                        pt[:, :BL], h_nat[:, hh * 128 : (hh + 1) * 128], ident[:BL, :BL]
                    )
                    nc.scalar.copy(out=ht[hh], in_=pt[:, :BL])

                for eh in range(2):
                    pt = pps.tile([128, 128], f32, tag="s", bufs=2, name="pt_v")
                    nc.tensor.transpose(
                        pt[:, :1], v_sb[:, eh * 128 : (eh + 1) * 128], ident[:1, :1]
                    )
                    nc.scalar.copy(out=vt16[eh], in_=pt[:, :1])

                # qb[eh][e, b] = sum_h WhT[h, e] * hT[h, b] + b_attn[e]
                for eh in range(2):
                    pq = pps.tile([128, 128], f32, tag="s", bufs=2, name="pt_q")
                    for hh in range(2):
                        nc.tensor.matmul(
                            pq[:, :BL],
                            wht[hh][:, eh * 128 : (eh + 1) * 128],
                            ht[hh],
                            start=(hh == 0),
                            stop=False,
                        )
                    nc.tensor.matmul(
                        pq[:, :BL],
                        b_attn_sb[:, eh * 128 : (eh + 1) * 128],
                        ones4,
                        start=False,
                        stop=True,
                    )
                    nc.scalar.copy(out=qb[eh], in_=pq[:, :BL])

            # ---------------- main loop ----------------
            pctx = [
                ppc.tile([2, 512], f32, tag=f"ctx{h}", name=f"pctx{h}")
                for h in range(2)
            ]

            def ctx_group(g):
                for half in range(2):
                    for jl in range(4):
                        n = g * 4 + jl
                        nc.tensor.matmul(
                            pctx[half],
                            u_g[g][:, 2 * half : 2 * half + 2, jl],
                            x_res[g][:, jl, half * 512 : (half + 1) * 512],
                            start=(n == 0),
                            stop=(n == NG * 4 - 1),
                        )

            for g in range(NG):
                xt_t = xtp.tile([128, 4096], bf16, tag="xt", name="xt")
                if g == 0:
                    for half in range(2):
                        tp = nc.sync.dma_start_transpose(
                            xt_t[:, half * 2048 : (half + 1) * 2048].rearrange(
                                "p (grp s) -> p grp s", s=128
                            ),
                            x_res[g][:, 2 * half : 2 * half + 2].rearrange(
                                "p a q -> p (a q)"
                            ),
                        )
                        tpose_deps.append((tp.ins.name, [read_of_group[g][half]]))
                        tpose_of_group.setdefault(g, []).append(tp.ins.name)
                else:
                    tp = nc.sync.dma_start_transpose(
                        xt_t.rearrange("p (grp s) -> p grp s", s=128),
                        x_res[g].rearrange("p a q -> p (a q)"),
                    )
                    tpose_deps.append((tp.ins.name, read_of_group[g]))
                    tpose_of_group.setdefault(g, []).append(tp.ins.name)
                xt_v = xt_t.rearrange("p (jl c s) -> p c jl s", jl=4, s=128)

                # context for the previous group first: its data is ready, so
                # it never head-of-line blocks the Tensor FIFO.
                if g >= 1:
                    ctx_group(g - 1)

                strip = pps.tile([97, 512], f32, tag="s", name="strip")
                for eh in range(2):
                    pe_t = [
                        ppe.tile([128, 512], f32, tag="e", name=f"pe{b}")
                        for b in range(BL)
                    ]
                    # stationary We^T chunk reused across the 4 batches
                    for hh in range(2):
                        for b in range(BL):
                            nc.tensor.matmul(
                                pe_t[b],
                                wet16[hh][:, eh * 128 : (eh + 1) * 128],
                                xt_v[:, b * 2 + hh],
                                start=(hh == 0),
                                stop=(hh == 1),
                            )
                    th_eh = []
                    for b in range(BL):
                        th = thp.tile([128, 512], bf16, tag="th", name="th")
                        nc.scalar.activation(
                            out=th,
                            in_=pe_t[b],
                            func=AF.Tanh,
                            bias=qb[eh][:, b : b + 1],
                        )
                        th_eh.append(th)
                    for b in range(BL):
                        nc.tensor.matmul(
                            strip[32 * b : 32 * b + 1, :],
                            vt16[eh],
                            th_eh[b],
                            start=(eh == 0),
                            stop=(eh == 1),
                            tile_position=(0, 32 * b),
                        )

                nc.scalar.activation(
                    out=st_g[g],
                    in_=strip,
                    func=AF.Exp,
                    accum_out=acc_all[:, g : g + 1],
                )
                for c in range(4):
                    pt = pps.tile([128, 256], bf16, tag="s", bufs=2, name="pt_u")
                    nc.tensor.transpose(
                        pt[:, :97],
                        st_g[g][:, c * 128 : (c + 1) * 128],
                        ident16[:97, :97],
                    )
                    nc.vector.tensor_copy(
                        out=u_g[g][:, :, c],
                        in_=pt.rearrange("p (a r) -> p a r", r=32)[:, :4, 0],
                    )

            ctx_group(NG - 1)
            for half in range(2):
                csb = wp.tile([2, 512], f32, tag="csb", name=f"csb{half}")
                nc.scalar.copy(out=csb, in_=pctx[half])
                nc.sync.dma_start(out=ctxu_t.ap()[half], in_=csb)

            accs = wp.tile([97, 1], f32)
            nc.vector.reduce_sum(out=accs, in_=acc_all, axis=mybir.AxisListType.X)
            nc.sync.dma_start(out=den_t.ap(), in_=accs)

    pace_deps = {
        read_of_group[g][0]: [tpose_of_group[g - 2][0]] for g in range(2, NG)
    }
    _dma_surgery(
        nc, mybir, read_names, tpose_deps, setup_read_names,
        pace_deps=pace_deps, verbose=verbose,
    )
    _split_multiwait(nc, mybir)
    return nc


def kernel(**inputs):
    from concourse.bass_utils import run_bass_kernel_spmd

    hidden = np.asarray(inputs["hidden"], dtype=np.float32)
    enc = np.asarray(inputs["encoder_outputs"], dtype=np.float32)
    w_attn = np.ascontiguousarray(np.asarray(inputs["W_attn"], dtype=np.float32))
    b_attn = np.ascontiguousarray(np.asarray(inputs["b_attn"], dtype=np.float32))
    w_v = np.ascontiguousarray(np.asarray(inputs["W_v"], dtype=np.float32))

    if "nc" not in _CACHE:
        _CACHE["nc"] = _build()
    nc = _CACHE["nc"]

    in_maps = []
    for c in range(NCORES):
        sl = slice(c * BL, (c + 1) * BL)
        in_maps.append(
            {
                "hidden": np.ascontiguousarray(hidden[sl]),
                "enc": np.ascontiguousarray(enc[:, sl, :]),
                "w_attn": w_attn,
                "b_attn": b_attn,
                "w_v": w_v,
            }
        )

    trace = bool(_CACHE.get("trace", False))
    res = run_bass_kernel_spmd(nc, in_maps, core_ids=list(range(NCORES)), trace=trace)
    _CACHE["last_results"] = res

    out = np.empty((1, B, H), dtype=np.float32)
    for c in range(NCORES):
        ctxu = res.results[c]["ctxu"]  # [2, 2, 512]
        den = res.results[c]["den"]    # [97, 1]
        for b in range(BL):
            half, row = b // 2, b % 2
            vals = ctxu[half, row, row * 256 : row * 256 + 256]
            out[0, c * BL + b] = vals / den[32 * b, 0]
    return out
